# revision 44
# baseline (speedup 1.0000x reference)
"""Llama-style GQA attention (B=1, S=2048, D=4096, 32 q-heads / 8 kv-heads,
rope, causal) on 8 trn2 NeuronCores, tensor-parallel over heads.

Core c owns q-heads 4c..4c+3 and kv-head c. Activations live in
"transposed" (feature-on-partition, seq-on-free) layout so every matmul
contracts over the partition dim. W_O is row-sharded; each core emits a
partial (D, S) bf16 output and the host sums the 8 partials and transposes.

v2: Q-proj and K-proj run in fp8 e4m3 with DoubleRow double-pumping
(2 contraction k-tiles per matmul pass, ~1.5x measured): softmax
normalization forgives the absolute score error this introduces
(measured 9e-3 end-to-end vs the 2e-2 budget; V/PV/O in fp8 measure
2.8e-2 and stay bf16). x is host-sent twice: an 8MB fp8 pair-packed
copy (resident in SBUF, feeds Q g0/g1 and K passes with zero
re-streaming) and the 16MB bf16 copy streamed once for V. The fp8
scale (x*16, w*2048) is folded out of the rope cos/sin tables.

Pass structure (PSUM is 8 banks; each head's (128,2048) f32 accumulator
is 4 banks): [g0: heads 0+1] -> [g1 head 2 + V kt 0..15] -> [g1 head 3 +
V kt 16..31] -> [K, with V's XBAR-transpose DMAs hidden under it].

RoPE trick: wq/wk rows are de-interleaved per head on the host
([0,2,..,126,1,3,..,127]) so the on-device pair (2j, 2j+1) becomes
(j, j+64) — a 64-partition block swap done with two partition-offset
vector ops against host-precomputed sign-folded cos/sin tables. The
permutation cancels in Q.K, and V/W_O are untouched by it.

Softmax is computed without max-subtraction (scores are bounded by
construction: |s| < ~10 => exp is safe in fp32), scoresT is held
(k on partitions, q on free) so P needs no transpose for P@V.
All PSUM tiles are uniform 2-bank (128,1024) tiles; softmax exp runs on
score PAIRS with one ACTIVATE; den = pair adds + ones-column matmul;
the normalize chain is software-pipelined one (h,j) block behind.
"""
import os
import numpy as np
import ml_dtypes

S = 2048
D = 4096
HD = 128
NCH = 4          # 512-wide seq chunks
KTILES = 32      # contraction tiles over D
NPAIR = 16       # DoubleRow k-tile pairs over D
CH = 512
SCALE = 1.0 / np.sqrt(128.0)
XS = 16.0        # fp8 scale on x
WS = 2048.0      # fp8 scale on wq/wk

_cache = {}


def _build():
    import concourse.bacc as bacc
    import concourse.tile as tile
    import concourse.mybir as mybir
    from concourse import bass

    dt = mybir.dt
    nc = bacc.Bacc("TRN2", target_bir_lowering=False, debug=False,
                   enable_asserts=False, num_devices=8)

    def inp(name, shape, d):
        return nc.dram_tensor(name, shape, d, kind="ExternalInput").ap()

    xT = inp("xT", (D, S), dt.bfloat16)          # bf16 x, streamed once (V)
    # fp8 x pair-packed: col = pair*4096 + u*2048 + s  (k-tile = 2*pair+u)
    xp8 = inp("xp8", (HD, 16 * 4096), dt.float8e4)
    # fp8 wq DoubleRow-packed: col = g*8192 + pr*512 + mi*256 + u*128 + m
    wqp8 = inp("wqp8", (HD, 16384), dt.float8e4)
    # fp8 wk: col = pr*256 + u*128 + m
    wkp8 = inp("wkp8", (HD, 4096), dt.float8e4)
    wvp = inp("wvp", (HD, 4096), dt.bfloat16)    # col = k*128 + m (ascending)
    wop = inp("wop", (HD, 16384), dt.bfloat16)   # col = of*512 + cf*128 + m
    cosd = inp("cosd", (HD, S), dt.bfloat16)     # scaled by 1/(XS*WS)
    sind = inp("sind", (HD, S), dt.bfloat16)
    trid = inp("trid", (HD, HD), dt.bfloat16)    # tri[k,c] = (c >= k)
    onesc = inp("onesc", (HD, 1), dt.bfloat16)
    outT = nc.dram_tensor("outT", (D, S), dt.bfloat16, kind="ExternalOutput").ap()

    f32 = dt.float32
    bf16 = dt.bfloat16
    fp8 = dt.float8e4
    Exp = mybir.ActivationFunctionType.Exp
    DR = mybir.MatmulPerfMode.DoubleRow

    with tile.TileContext(nc) as tc:
        with (
            tc.tile_pool(name="const", bufs=1) as constp,
            tc.tile_pool(name="xs", bufs=7) as xpool,
            tc.tile_pool(name="wq", bufs=3) as wqpool,
            tc.tile_pool(name="wq1", bufs=8) as wq1pool,
            tc.tile_pool(name="wkv", bufs=2) as wkvpool,
            tc.tile_pool(name="wo", bufs=2) as wopool,
            tc.tile_pool(name="acts", bufs=1) as actp,
            tc.tile_pool(name="pt", bufs=5) as ptpool,
            tc.tile_pool(name="ds", bufs=4) as dspool,
            tc.tile_pool(name="tmp", bufs=2) as tmpp,
            tc.tile_pool(name="ost", bufs=2) as ostp,
            tc.tile_pool(name="ps", bufs=4, space="PSUM") as psp,
        ):
            # ---- resident fp8 x pairs. Pairs 0-3: single-pair tiles
            # [128, 2, 2048] (fine arrival granularity at cold start);
            # pairs 4-15: two-pair super-tiles [128, 2, 2, 2048] loaded as
            # one 1MB DMA each (>=1MiB transfers run at ~75% of DMA peak,
            # 512KB noticeably less).
            x8 = [actp.tile([HD, 2, S], fp8, tag=f"x8_{p}", name=f"x8_{p}")
                  for p in range(4)]
            x8s = [actp.tile([HD, 4, 2, S], fp8, tag=f"x8s_{s}", name=f"x8s_{s}")
                   for s in range(3)]

            def x8sl(p, c0, c1):
                if p < 4:
                    return x8[p][:, :, c0:c1]
                s, hp = (p - 4) // 4, (p - 4) % 4
                return x8s[s][:, hp, :, c0:c1]

            # Startup: first wq8 chunk halves on sync; x8 pair 0 quarters on
            # gpsimd, pair 1 on scalar; then ascending pairs cycling queues
            # with wq chunks interleaved by need-time on sync.
            def wq_chunk_g0(t):
                # chunk t covers DR pairs 2t,2t+1: [128, (pq,mi)=4, u=2, m=128]
                wt = wqpool.tile([HD, 4, 2, HD], fp8, tag="wq", name=f"wqc0_{t}")
                return wt

            wq_pre = {}
            wt0 = wq_chunk_g0(0)
            # startup on the two HWDGE queues (SWDGE prep is ~630ns/desc):
            # first DR matmul needs wq pair-0 weights + x8 pair-0 cols 0:1024.
            nc.sync.dma_start(wt0[:, 0:2, :, :], wqp8[:, 0:512])
            nc.sync.dma_start(x8[0][:, 0, 0:2 * CH], xp8[:, 0:2 * CH])
            nc.scalar.dma_start(x8[0][:, 1, 0:2 * CH], xp8[:, S:S + 2 * CH])
            nc.sync.dma_start(x8[0][:, 0, 2 * CH:S], xp8[:, 2 * CH:S])
            nc.scalar.dma_start(x8[0][:, 1, 2 * CH:S], xp8[:, S + 2 * CH:2 * S])
            nc.sync.dma_start(wt0[:, 2:4, :, :], wqp8[:, 512:1024])
            wq_pre[0] = wt0
            for u in range(2):
                nc.scalar.dma_start(x8[1][:, u, :],
                                    xp8[:, 4096 + u * S:4096 + (u + 1) * S])
            wt1 = wq_chunk_g0(1)
            nc.sync.dma_start(wt1[:], wqp8[:, 1024:2048])
            wq_pre[1] = wt1
            qs = [nc.gpsimd, nc.scalar, nc.sync]
            nc.gpsimd.dma_start(x8[2][:], xp8[:, 2 * 4096:3 * 4096])
            wt2 = wq_chunk_g0(2)
            nc.sync.dma_start(wt2[:], wqp8[:, 2048:3072])
            wq_pre[2] = wt2
            nc.scalar.dma_start(x8[3][:], xp8[:, 3 * 4096:4 * 4096])
            for s in range(3):       # 2MB four-pair supers, need at pair 4+4s
                eng = qs[s % 3]
                eng.dma_start(x8s[s][:],
                              xp8[:, (4 + 4 * s) * 4096:(8 + 4 * s) * 4096])
                for t in (3 + 2 * s, 4 + 2 * s):
                    if t <= 7 and t not in wq_pre:
                        wt = wq_chunk_g0(t)
                        nc.sync.dma_start(wt[:],
                                          wqp8[:, t * 1024:(t + 1) * 1024])
                        wq_pre[t] = wt
            for t in range(8):
                if t not in wq_pre:
                    wt = wq_chunk_g0(t)
                    nc.sync.dma_start(wt[:], wqp8[:, t * 1024:(t + 1) * 1024])
                    wq_pre[t] = wt
            xq = [0]

            # ---- constants: tiles now, DMAs emitted after g0's stream
            cos_t = constp.tile([HD, S], bf16, tag="cos")
            sin_t = constp.tile([HD, S], bf16, tag="sin")
            tri_t = constp.tile([HD, HD], bf16, tag="tri")
            onesc_t = constp.tile([HD, 1], bf16, tag="onesc")

            # persistent activations (bf16, feature x seq)
            qtr = [actp.tile([HD, S], bf16, tag=f"qtr{h}", name=f"qtr{h}") for h in range(4)]
            ktr = actp.tile([HD, S], bf16, tag="ktr")
            vbuf = actp.tile([HD, 16 * HD], bf16, tag="vbuf")  # (k 128, kt*128 d)
            ctxn = [actp.tile([HD, S], bf16, tag=f"ctx{h}", name=f"ctx{h}") for h in range(4)]

            def psum2(name):
                return psp.tile([HD, 2 * CH], f32, tag="mm2", name=name)

            def rope_into(dst, ps, ch):
                """dst[:, ch*512:...] (bf16) = st*COS + swap64(st)*SIN"""
                c0 = ch * CH
                st = tmpp.tile([HD, CH], bf16, tag="rst", bufs=6)
                nc.scalar.copy(st[:], ps)      # frees the PSUM half quickly
                t1 = tmpp.tile([HD, CH], bf16, tag="r1")
                nc.vector.tensor_mul(t1[:], st[:], cos_t[:, c0:c0 + CH])
                t2 = tmpp.tile([HD, CH], bf16, tag="r2")
                nc.vector.tensor_mul(t2[0:64, :], st[64:128, :], sin_t[64:128, c0:c0 + CH])
                nc.vector.tensor_mul(t2[64:128, :], st[0:64, :], sin_t[0:64, c0:c0 + CH])
                nc.vector.tensor_add(dst[:, c0:c0 + CH], t1[:], t2[:])

            # ---- Q g0: heads 0,1 in fp8 DoubleRow over 16 k-pairs ----
            qps = [[psum2(f"qps0_{mi}_{cp}") for cp in range(2)]
                   for mi in range(2)]
            # HAM warmup: dependency-free matmuls during the startup DMA
            # window so the PE clock-gate opens before the first real matmul.
            scr = constp.tile([HD, 64], bf16, tag="scr")
            nc.vector.memset(scr[:], 0)
            for w in range(100):
                nc.tensor.matmul(qps[0][0][0:64, 0:64], scr[:], scr[:],
                                 start=True, stop=True)
            for pr in range(NPAIR):
                t = pr // 2
                pq = pr % 2
                wqc = wq_pre[t]
                for mi in range(2):
                    lhs = wqc[:, pq * 2 + mi, :, :]           # [128, 2, 128]
                    for ch in range(NCH):
                        nc.tensor.matmul(
                            qps[mi][ch // 2][:, (ch % 2) * CH:(ch % 2 + 1) * CH],
                            lhs, x8sl(pr, ch * CH, (ch + 1) * CH),
                            start=(pr == 0), stop=(pr == NPAIR - 1),
                            perf_mode=DR)
            nc.sync.dma_start(cos_t[:], cosd[:])
            nc.sync.dma_start(sin_t[:], sind[:])
            nc.scalar.dma_start(tri_t[:], trid[:])
            nc.scalar.dma_start(onesc_t[:], onesc[:])
            for mi in range(2):
                for ch in range(NCH):
                    rope_into(qtr[mi],
                              qps[mi][ch // 2][:, (ch % 2) * CH:(ch % 2 + 1) * CH],
                              ch)

            # ---- pass2: Q g1 (heads 2,3 one at a time) + V (bf16 streamed)
            # wq g1 chunks stay resident across both sub-passes.
            # wv as one 1MB 8KB-row DMA; wq g1 as two 512KB 4KB-row DMAs
            # (small-row transfers are descriptor-dominated on the saturated
            # front-half DMA window)
            wvt = wkvpool.tile([HD, 4096], bf16, tag="wv", name="wvt", bufs=1)
            nc.sync.dma_start(wvt[:], wvp[:])
            wq1 = []
            for t in range(2):
                wt = wq1pool.tile([HD, 16, 2, HD], fp8, tag="wq1",
                                  name=f"wqc1_{t}", bufs=2)
                eng = [nc.gpsimd, nc.scalar][t]
                eng.dma_start(wt[:], wqp8[:, 8192 + t * 4096:8192 + (t + 1) * 4096])
                wq1.append(wt)

            vps = [psum2(f"vps{cp}") for cp in range(2)]

            def get_xbf(k, engs):
                """streamed bf16 x k-tile as 2 half tiles; returns slicer"""
                halves = []
                for u2 in range(2):
                    tt = xpool.tile([HD, 2 * CH], bf16, tag="xt",
                                    name=f"xt{k}_{u2}")
                    eng = engs[xq[0] % len(engs)]
                    xq[0] += 1
                    eng.dma_start(tt[:], xT[k * HD:(k + 1) * HD,
                                            u2 * 2 * CH:(u2 + 1) * 2 * CH])
                    halves.append(tt)
                return lambda ch, hs=halves: hs[ch // 2][:, (ch % 2) * CH:
                                                         (ch % 2 + 1) * CH]

            for mi in range(2):
                qps1 = [psum2(f"qps1_{mi}_{cp}") for cp in range(2)]
                for pr in range(NPAIR):
                    # V matmul for kt = mi*16 + pr (bf16)
                    kt = mi * NPAIR + pr
                    xsl = get_xbf(kt, qs)
                    lhs = wq1[pr // 8][:, (pr % 8) * 2 + mi, :, :]
                    for ch in range(NCH):
                        nc.tensor.matmul(
                            qps1[ch // 2][:, (ch % 2) * CH:(ch % 2 + 1) * CH],
                            lhs, x8sl(pr, ch * CH, (ch + 1) * CH),
                            start=(pr == 0), stop=(pr == NPAIR - 1),
                            perf_mode=DR)
                    for ch in range(NCH):
                        nc.tensor.matmul(
                            vps[ch // 2][:, (ch % 2) * CH:(ch % 2 + 1) * CH],
                            wvt[:, kt * HD:(kt + 1) * HD], xsl(ch),
                            start=(kt == 0), stop=(kt == KTILES - 1))
                for ch in range(NCH):
                    rope_into(qtr[2 + mi],
                              qps1[ch // 2][:, (ch % 2) * CH:(ch % 2 + 1) * CH],
                              ch)

            # ---- pass3: K (fp8 DoubleRow, resident x8); V transpose hidden
            # V: stage bf16, then DMA-XBAR block transpose into (seq, d)
            # all transposes ride the SYNC queue: putting any on scalar delays
            # the attention entry exp (the ACT sequencer issues them serially)
            vstage = actp.tile([HD, S], bf16, tag="vstage")
            for ch in range(NCH):
                nc.vector.tensor_copy(
                    vstage[:, ch * CH:(ch + 1) * CH],
                    vps[ch // 2][:, (ch % 2) * CH:(ch % 2 + 1) * CH])
                for st in range(4 * ch, 4 * ch + 4):
                    nc.sync.dma_start_transpose(
                        vbuf[:, st * HD:(st + 1) * HD],
                        vstage[:, st * HD:(st + 1) * HD])

            # K split by seq-half: kps[0] (ch 0,1) completes first so its rope
            # (DVE) overlaps the second half's matmuls and attention can start
            # as soon as ktr ch0 is ready.
            wkt = wkvpool.tile([HD, 16, 2, HD], fp8, tag="wk", name="wkt", bufs=1)
            nc.gpsimd.dma_start(wkt[:], wkp8[:])
            krope23 = [None]
            for cp in range(2):
                kpsc = psum2(f"kps{cp}")
                for pr in range(NPAIR):
                    for ch in (2 * cp, 2 * cp + 1):
                        nc.tensor.matmul(
                            kpsc[:, (ch % 2) * CH:(ch % 2 + 1) * CH],
                            wkt[:, pr, :, :],
                            x8sl(pr, ch * CH, (ch + 1) * CH),
                            start=(pr == 0), stop=(pr == NPAIR - 1),
                            perf_mode=DR)
                if cp == 0:
                    for ch in (0, 1):
                        rope_into(ktr, kpsc[:, ch * CH:(ch + 1) * CH], ch)
                else:
                    # ch2/3 rope is only needed by j>=2 blocks: drain the PSUM
                    # eagerly (scalar copy) but defer the DVE mul/add past the
                    # first attention block so the entry exp->mask->PV chain
                    # isn't queued behind it on the vector engine.
                    sts = []
                    for ch in (2, 3):
                        st = tmpp.tile([HD, CH], bf16, tag="rst", bufs=6,
                                       name=f"kst{ch}")
                        nc.scalar.copy(st[:],
                                       kpsc[:, (ch % 2) * CH:(ch % 2 + 1) * CH])
                        sts.append(st)
                    def krope23_fn(sts=sts):
                        for ch, st in zip((2, 3), sts):
                            c0 = ch * CH
                            t1 = tmpp.tile([HD, CH], bf16, tag="r1")
                            nc.vector.tensor_mul(t1[:], st[:],
                                                 cos_t[:, c0:c0 + CH])
                            t2 = tmpp.tile([HD, CH], bf16, tag="r2")
                            nc.vector.tensor_mul(t2[0:64, :], st[64:128, :],
                                                 sin_t[64:128, c0:c0 + CH])
                            nc.vector.tensor_mul(t2[64:128, :], st[0:64, :],
                                                 sin_t[0:64, c0:c0 + CH])
                            nc.vector.tensor_add(ktr[:, c0:c0 + CH],
                                                 t1[:], t2[:])
                    krope23[0] = krope23_fn

            # ---- attention, per head / q-chunk (paired k-tiles) ----
            # finalize (den matmul + reciprocal + broadcast + normalize) is
            # pipelined one (h,j) behind so its latency hides under the next
            # block's matmuls and the PSUM ring never waits on it.
            def finalize(fin):
                cd, h, j = fin
                recip = tmpp.tile([1, CH], f32, tag="recip")
                nc.vector.reciprocal_approx_fast(recip[:], cd[0:1, CH:2 * CH])
                bcs = tmpp.tile([HD, CH], f32, tag="bcs")
                nc.gpsimd.partition_broadcast(bcs[:], recip[:], channels=HD)
                nc.vector.tensor_mul(ctxn[h][:, j * CH:(j + 1) * CH],
                                     cd[:, 0:CH], bcs[:])

            # Diagonal-block handling: the 4 diagonal k-tiles of q-chunk j
            # (kt = 4j+m) only have live q-columns [128m, 512).  Scores are
            # packed COMPACTLY (ptA = m0 full at [0:512) + m1 at [512:896);
            # ptB = m2 at [0:256) + m3 at [256:384)) so exp runs on 896+384
            # cols instead of 2048, the causal mask shrinks to 4 (128,128)
            # triangle muls, and score/PV matmuls skip fully-dead columns.
            # PV per column-region: the LAST writer needs stop=True, so for
            # j>0 the full-width m0 PV is issued last; j==0 (where m0 also
            # carries start) uses 7 region-split pieces.
            # Emission is a cross-block software pipeline: per block, P-steps
            # produce pt tiles (scores+exp+mask) and C-steps consume them
            # (PV matmuls + DVE den folds) with a 1-step lag; the diagonal PV
            # carries into the NEXT block, and the block's den matmuls (which
            # depend on DVE folds) are deferred into the next block's stream
            # so they never head-of-line-block the in-order PE queue.
            pend_q = []       # deferred [ready_step, closure] (den mms, finalize)
            gstep = [0]       # global P-step counter (drain-lag bookkeeping)
            carry = [None]    # diag-PV closure carried across blocks
            carry_at = [1]    # P-step index where the carry fires
            for h in range(4):
                for j in range(NCH):
                    q0 = j * CH
                    nd = 2 * j                 # non-diagonal pairs
                    ctxden = psum2(f"cd{h}_{j}")   # ctx in half0, den row half1
                    pts = []     # one (128,1024) bf16 tile per k-tile pair
                    def score_exp_pair(pr, j=j, h=h, q0=q0, pts=pts):
                        sps = psum2(f"sps{h}_{j}_{pr}")
                        for u in range(2):
                            kt = 2 * pr + u
                            nc.tensor.matmul(sps[:, u * CH:(u + 1) * CH],
                                             ktr[:, kt * HD:(kt + 1) * HD],
                                             qtr[h][:, q0:q0 + CH],
                                             start=True, stop=True)
                        pt = ptpool.tile([HD, 2 * CH], bf16, tag="pt",
                                         name=f"pt{h}_{j}_{pr}")
                        nc.scalar.activation(pt[:], sps[:], Exp, scale=SCALE)
                        pts.append(pt)
                    def pv_pair(pr, j=j, h=h, ctxden=ctxden, pts=pts):
                        for u in range(2):
                            kt = 2 * pr + u
                            nc.tensor.matmul(ctxden[:, 0:CH],
                                             vbuf[:, kt * HD:(kt + 1) * HD],
                                             pts[pr][:, u * CH:(u + 1) * CH],
                                             start=(kt == 0), stop=False)
                        if pr % 2 == 1:
                            gq = pr // 2
                            pa = dspool.tile([HD, 2 * CH], bf16, tag="da",
                                             bufs=1, name=f"da{h}_{j}_{gq}")
                            nc.vector.tensor_add(pa[:], pts[pr - 1][:],
                                                 pts[pr][:])
                            hs = dspool.tile([HD, CH], bf16, tag="dc", bufs=6,
                                             name=f"dh{h}_{j}_{gq}")
                            nc.vector.tensor_add(hs[:], pa[:, 0:CH],
                                                 pa[:, CH:2 * CH])
                            pend_q.append(
                                [gstep[0] + 3,
                                 lambda hs=hs, ctxden=ctxden, gq=gq:
                                 nc.tensor.matmul(ctxden[0:1, CH:2 * CH],
                                                  onesc_t[:], hs[:],
                                                  start=(gq == 0), stop=False)])

                    def diag_scores_A(j=j, h=h, q0=q0):
                        kb = 4 * j * HD
                        sps = psum2(f"spsA{h}_{j}")
                        nc.tensor.matmul(sps[:, 0:CH], ktr[:, kb:kb + HD],
                                         qtr[h][:, q0:q0 + CH],
                                         start=True, stop=True)
                        nc.tensor.matmul(sps[:, CH:CH + 384],
                                         ktr[:, kb + HD:kb + 2 * HD],
                                         qtr[h][:, q0 + HD:q0 + CH],
                                         start=True, stop=True)
                        ptA = ptpool.tile([HD, 2 * CH], bf16, tag="pt",
                                          name=f"ptA{h}_{j}")
                        nc.scalar.activation(ptA[:, 0:CH + 384],
                                             sps[:, 0:CH + 384], Exp, scale=SCALE)
                        nc.vector.tensor_mul(ptA[:, 0:HD], ptA[:, 0:HD], tri_t[:])
                        nc.vector.tensor_mul(ptA[:, CH:CH + HD],
                                             ptA[:, CH:CH + HD], tri_t[:])
                        return ptA
                    def diag_scores_B(j=j, h=h, q0=q0):
                        kb = (4 * j + 2) * HD
                        sps = psum2(f"spsB{h}_{j}")
                        nc.tensor.matmul(sps[:, 0:256], ktr[:, kb:kb + HD],
                                         qtr[h][:, q0 + 256:q0 + CH],
                                         start=True, stop=True)
                        nc.tensor.matmul(sps[:, 256:384],
                                         ktr[:, kb + HD:kb + 2 * HD],
                                         qtr[h][:, q0 + 384:q0 + CH],
                                         start=True, stop=True)
                        ptB = ptpool.tile([HD, 2 * CH], bf16, tag="pt",
                                          name=f"ptB{h}_{j}")
                        nc.scalar.activation(ptB[:, 0:384], sps[:, 0:384],
                                             Exp, scale=SCALE)
                        nc.vector.tensor_mul(ptB[:, 0:HD], ptB[:, 0:HD], tri_t[:])
                        nc.vector.tensor_mul(ptB[:, 256:384],
                                             ptB[:, 256:384], tri_t[:])
                        return ptB
                    def diag_pv_den(ptA, ptB, j=j, h=h, ctxden=ctxden):
                        kb = 4 * j * HD
                        v0 = vbuf[:, kb:kb + HD]
                        v1 = vbuf[:, kb + HD:kb + 2 * HD]
                        v2 = vbuf[:, kb + 2 * HD:kb + 3 * HD]
                        v3 = vbuf[:, kb + 3 * HD:kb + 4 * HD]
                        if j == 0:
                            nc.tensor.matmul(ctxden[:, 0:HD], v0, ptA[:, 0:HD],
                                             start=True, stop=True)
                            nc.tensor.matmul(ctxden[:, HD:CH], v0, ptA[:, HD:CH],
                                             start=True, stop=False)
                            nc.tensor.matmul(ctxden[:, HD:256], v1,
                                             ptA[:, CH:CH + HD],
                                             start=False, stop=True)
                            nc.tensor.matmul(ctxden[:, 256:CH], v1,
                                             ptA[:, CH + HD:CH + 384],
                                             start=False, stop=False)
                            nc.tensor.matmul(ctxden[:, 256:384], v2,
                                             ptB[:, 0:HD],
                                             start=False, stop=True)
                            nc.tensor.matmul(ctxden[:, 384:CH], v2,
                                             ptB[:, HD:256],
                                             start=False, stop=False)
                            nc.tensor.matmul(ctxden[:, 384:CH], v3,
                                             ptB[:, 256:384],
                                             start=False, stop=True)
                        else:
                            nc.tensor.matmul(ctxden[:, HD:CH], v1,
                                             ptA[:, CH:CH + 384],
                                             start=False, stop=False)
                            nc.tensor.matmul(ctxden[:, 256:CH], v2,
                                             ptB[:, 0:256],
                                             start=False, stop=False)
                            nc.tensor.matmul(ctxden[:, 384:CH], v3,
                                             ptB[:, 256:384],
                                             start=False, stop=False)
                            nc.tensor.matmul(ctxden[:, 0:CH], v0, ptA[:, 0:CH],
                                             start=False, stop=True)
                        # den: fold m0+m1 (q-aligned) into afold, m2+m3 into
                        # bfold; for j>0 merge bfold into afold (in-place) so
                        # one full-width ones-matmul closes the group.
                        afold = dspool.tile([HD, CH], bf16, tag="dc", bufs=6,
                                            name=f"af{h}_{j}")
                        nc.vector.tensor_copy(afold[:, 0:HD], ptA[:, 0:HD])
                        nc.vector.tensor_add(afold[:, HD:CH], ptA[:, HD:CH],
                                             ptA[:, CH:CH + 384])
                        bfold = dspool.tile([HD, 256], bf16, tag="db", bufs=4,
                                            name=f"bf{h}_{j}")
                        nc.vector.tensor_copy(bfold[:, 0:HD], ptB[:, 0:HD])
                        nc.vector.tensor_add(bfold[:, HD:256], ptB[:, HD:256],
                                             ptB[:, 256:384])
                        if j == 0:
                            pend_q.append(
                                [gstep[0] + 1,
                                 lambda ctxden=ctxden, afold=afold:
                                 nc.tensor.matmul(ctxden[0:1, CH:CH + 256],
                                                  onesc_t[:], afold[:, 0:256],
                                                  start=True, stop=True)])
                            pend_q.append(
                                [gstep[0] + 1,
                                 lambda ctxden=ctxden, afold=afold:
                                 nc.tensor.matmul(ctxden[0:1, CH + 256:2 * CH],
                                                  onesc_t[:], afold[:, 256:CH],
                                                  start=True, stop=False)])
                            pend_q.append(
                                [gstep[0] + 1,
                                 lambda ctxden=ctxden, bfold=bfold:
                                 nc.tensor.matmul(ctxden[0:1, CH + 256:2 * CH],
                                                  onesc_t[:], bfold[:],
                                                  start=False, stop=True)])
                        else:
                            nc.vector.tensor_add(afold[:, 256:CH],
                                                 afold[:, 256:CH], bfold[:])
                            pend_q.append(
                                [gstep[0] + 1,
                                 lambda ctxden=ctxden, afold=afold:
                                 nc.tensor.matmul(ctxden[0:1, CH:2 * CH],
                                                  onesc_t[:], afold[:],
                                                  start=False, stop=True)])

                    # --- emit this block's pipeline ---
                    res = {}
                    def carry_fn(f=diag_pv_den, res=res, cd=ctxden, h=h, j=j):
                        f(res['A'], res['B'])
                        pend_q.append([gstep[0] + 1,
                                       lambda: finalize((cd, h, j))])
                    P = ([(lambda pr=pr: score_exp_pair(pr)) for pr in range(nd)]
                         + [diag_scores_A, diag_scores_B])
                    for i, pstep in enumerate(P):
                        r = pstep()
                        if i == nd:
                            res['A'] = r
                        elif i == nd + 1:
                            res['B'] = r
                        gstep[0] += 1
                        if i == carry_at[0] and carry[0] is not None:
                            carry[0]()
                            carry[0] = None
                        if i >= 1:
                            ndrain = 2 if len(pend_q) > 2 else 1
                            for _ in range(ndrain):
                                if pend_q and pend_q[0][0] <= gstep[0]:
                                    pend_q.pop(0)[1]()
                            if i - 1 < nd:
                                pv_pair(i - 1)
                    carry[0] = carry_fn
                    # a j==0 block's exp-B lands late: fire its carried PV one
                    # step deeper into the next block
                    carry_at[0] = 2 if nd == 0 else 1
                    if h == 0 and j == 0 and krope23[0] is not None:
                        krope23[0]()
                        krope23[0] = None
            if carry[0] is not None:
                carry[0]()
            for _, fn in pend_q:
                fn()
            pend_q = []

            # ---- O projection (row-sharded W_O -> partial outT, bf16) ----
            # last two of-tiles run ch-outer/cf-inner so each seq-chunk's
            # accumulation closes early and its cast+store pipeline under the
            # remaining matmuls (shrinks the post-PE tail).
            for t in range(8):   # wo chunk = 4 of-tiles, one 512KB DMA
                woc = wopool.tile([HD, 2048], bf16, tag="wo", name=f"woc{t}")
                nc.scalar.dma_start(woc[:], wop[:, t * 2048:(t + 1) * 2048])
                for oo in range(4):
                    of = 4 * t + oo
                    ops = [psum2(f"ops{of}_{cp}") for cp in range(2)]
                    if of < 30:
                        for cf in range(4):
                            lhs = woc[:, oo * 512 + cf * HD:oo * 512 + (cf + 1) * HD]
                            for ch in range(NCH):
                                nc.tensor.matmul(
                                    ops[ch // 2][:, (ch % 2) * CH:(ch % 2 + 1) * CH],
                                    lhs, ctxn[cf][:, ch * CH:(ch + 1) * CH],
                                    start=(cf == 0), stop=(cf == 3))
                        ost = ostp.tile([HD, S], bf16, tag="ost", name=f"ost{of}")
                        for ch in range(NCH):
                            nc.vector.tensor_copy(
                                ost[:, ch * CH:(ch + 1) * CH],
                                ops[ch // 2][:, (ch % 2) * CH:(ch % 2 + 1) * CH])
                        oeng = nc.sync if of % 2 == 0 else nc.scalar
                        oeng.dma_start(outT[of * HD:(of + 1) * HD, :], ost[:])
                    else:
                        ost = ostp.tile([HD, S], bf16, tag="ost", name=f"ost{of}")
                        for ch in range(NCH):
                            for cf in range(4):
                                lhs = woc[:, oo * 512 + cf * HD:oo * 512 + (cf + 1) * HD]
                                nc.tensor.matmul(
                                    ops[ch // 2][:, (ch % 2) * CH:(ch % 2 + 1) * CH],
                                    lhs, ctxn[cf][:, ch * CH:(ch + 1) * CH],
                                    start=(cf == 0), stop=(cf == 3))
                            src = ops[ch // 2][:, (ch % 2) * CH:(ch % 2 + 1) * CH]
                            dst = ost[:, ch * CH:(ch + 1) * CH]
                            if ch % 2 == 1:
                                nc.scalar.copy(dst, src)
                            else:
                                nc.vector.tensor_copy(dst, src)
                            if of == 31 and ch == 3:   # final chunk: halve it
                                nc.sync.dma_start(
                                    outT[of * HD:(of + 1) * HD,
                                         ch * CH:ch * CH + 256],
                                    ost[:, ch * CH:ch * CH + 256])
                                nc.scalar.dma_start(
                                    outT[of * HD:(of + 1) * HD,
                                         ch * CH + 256:(ch + 1) * CH],
                                    ost[:, ch * CH + 256:(ch + 1) * CH])
                            else:
                                oeng = nc.sync if ch % 2 == 0 else nc.scalar
                                oeng.dma_start(
                                    outT[of * HD:(of + 1) * HD,
                                         ch * CH:(ch + 1) * CH],
                                    ost[:, ch * CH:(ch + 1) * CH])

    nc.compile()
    return nc


def _host_inputs(x, wq, wk, wv, wo):
    bf16 = ml_dtypes.bfloat16
    e4 = ml_dtypes.float8_e4m3
    perm = np.concatenate([np.arange(0, 128, 2), np.arange(1, 128, 2)])
    half = 64
    inv = 1.0 / (10000.0 ** (np.arange(half) / half))
    ang = np.arange(S)[:, None] * inv[None, :]
    rsc = 1.0 / (XS * WS)
    cosd = np.ascontiguousarray(
        np.concatenate([np.cos(ang).T, np.cos(ang).T], 0) * rsc).astype(bf16)
    sind = np.ascontiguousarray(
        np.concatenate([np.sin(ang).T, -np.sin(ang).T], 0) * rsc).astype(bf16)
    trid = (np.arange(HD)[None, :] >= np.arange(HD)[:, None]).astype(bf16)
    onescol = np.ones((HD, 1), bf16)
    xTb = np.ascontiguousarray(x[0].T).astype(bf16)
    # fp8 x pairs: xp8[p, pair*4096 + u*2048 + s] = 16*x[s, (2pair+u)*128+p]
    x8full = (x[0].T * XS).astype(e4)              # (D, S)
    xp8 = np.ascontiguousarray(
        x8full.reshape(16, 2, HD, S).transpose(2, 0, 1, 3).reshape(HD, 16 * 4096))

    in_maps = []
    for c in range(8):
        qrows = slice(512 * c, 512 * (c + 1))
        wq_c = wq[qrows].reshape(4, HD, D)[:, perm].reshape(512, D)
        # wqp8[p, g*8192 + pr*512 + mi*256 + u*128 + m]
        #   = WS*wq_c[g*256 + mi*128 + m, (pr*2+u)*128 + p]
        wq6 = (wq_c * WS).astype(e4).reshape(2, 2, HD, NPAIR, 2, HD)
        wqp8 = np.ascontiguousarray(
            wq6.transpose(5, 0, 3, 1, 4, 2).reshape(HD, 16384))
        wk_c = wk[HD * c:HD * (c + 1)][perm]
        # wkp8[p, pr*256 + u*128 + m] = WS*wk_c[m, (pr*2+u)*128 + p]
        wk5 = (wk_c * WS).astype(e4).reshape(HD, NPAIR, 2, HD)
        wkp8 = np.ascontiguousarray(
            wk5.transpose(3, 1, 2, 0).reshape(HD, 4096))
        wv_c = wv[HD * c:HD * (c + 1)]
        # wvp[p, k*128 + m] = wv_c[m, k*128+p], ascending k
        wvp = np.ascontiguousarray(
            wv_c.reshape(HD, KTILES, HD).transpose(2, 1, 0)
            .reshape(HD, 4096)).astype(bf16)
        woT_c = wo[:, qrows].T            # (512 ctx-feat, 4096 out-feat)
        # wop[p, of*512 + cf*128 + m] = woT_c[cf*128+p, of*128+m]
        wop = np.ascontiguousarray(
            woT_c.reshape(4, HD, KTILES, HD).transpose(1, 2, 0, 3)
            .reshape(HD, 16384)).astype(bf16)
        in_maps.append({
            "xT": xTb, "xp8": xp8, "wqp8": wqp8, "wkp8": wkp8, "wvp": wvp,
            "wop": wop, "cosd": cosd, "sind": sind, "trid": trid,
            "onesc": onescol,
        })
    return in_maps


LAST_RESULTS = None


def kernel(x, wq, wk, wv, wo, attn_mask):
    global LAST_RESULTS
    from concourse import bass_utils
    if "nc" not in _cache:
        _cache["nc"] = _build()
    nc = _cache["nc"]
    in_maps = _host_inputs(np.asarray(x, np.float32), np.asarray(wq, np.float32),
                           np.asarray(wk, np.float32), np.asarray(wv, np.float32),
                           np.asarray(wo, np.float32))
    res = bass_utils.run_bass_kernel_spmd(
        nc, in_maps, core_ids=list(range(8)),
        trace=bool(os.environ.get("BASS_TRACE")))
    LAST_RESULTS = res
    acc = res.results[0]["outT"].astype(np.float64)
    for c in range(1, 8):
        acc = acc + res.results[c]["outT"].astype(np.float64)
    return np.ascontiguousarray(acc.T).astype(np.float32).reshape(1, S, D)
